# revision 5
# baseline (speedup 1.0000x reference)
"""Trainium2 Bass kernel for nn_BodyKinematics (batched tree forward
kinematics).

Contract: kernel(**inputs) takes the FULL unsharded inputs as numpy arrays and
returns the FULL output (B, N, 4, 4) float32.  The batch dim is sharded across
8 NeuronCores (pure data parallelism); per-edge parameters are replicated.

Device design ("quad layout"):
  Each of the 128 SBUF partitions owns 4 batch rows (b4 = innermost free-dim
  index), so one op set covers the core's whole 512-row shard and every DVE
  operand has a packed innermost dim -> fp16 tensor ops run in 2x/4x modes.
    theta = tanh(la) (ACT), sin/cos via ACT Sin (cos through |t|)
    local = Rx@Ry@Rz@tip via 6 Givens triples over (e, l, b4) tiles, fp16,
            split DVE/POOL by edge range; tip rows host-expanded along b4
    tree  = level-parallel parent@child on (n, i, l, b4) fp16 tile
    out   = 32 (b4, 32-node) chunks converted f16->f32 (ACT/DVE) and DMA'd
  Constant bottom rows (0,0,0,1) are initialized once outside the timed loop.
"""

import os
import sys

for _p in ("/opt/trn_rl_repo",):
    if _p not in sys.path and os.path.isdir(_p):
        sys.path.insert(0, _p)

import numpy as np

B, E, N = 4096, 255, 256
J = 3 * E
NCORE, P, B4 = 8, 128, 4
BPC = P * B4      # 512 batch rows per core
PI = float(np.pi)

# engine/order knobs (chosen via TimelineSim sweeps)
BC_STAGES = [("v", 0, 127), ("g", 205, 255), ("v", 127, 205)]
CONV_ENG = ["a", "a", "a", "a", "a", "a", "v", "v"]   # keyed by node-block
OB_BUFS = 6

_state: dict = {}


# --------------------------------------------------------------------------- #
# numpy fallback (exact float32 port of the reference) — used only if the
# inputs don't match the structure the device kernel was built for.
# --------------------------------------------------------------------------- #
def _np_skew(a):
    x, y, z = a[..., 0], a[..., 1], a[..., 2]
    zero = np.zeros_like(x)
    return np.stack([
        np.stack([zero, -z, y], -1),
        np.stack([z, zero, -x], -1),
        np.stack([-y, x, zero], -1)], -2)


def _np_fallback(log_angles, tip_to_base, rot_axes, rot_constraints):
    la = log_angles.astype(np.float32)
    b, e3 = la.shape
    e = e3 // 3
    n = e + 1
    theta = np.tanh(la) * rot_constraints[:, 0] + rot_constraints[:, 1]
    K = _np_skew(rot_axes.astype(np.float32))
    K2 = np.einsum('mij,mjk->mik', K, K).astype(np.float32)
    s = np.sin(theta)[..., None, None]
    c = (1.0 - np.cos(theta))[..., None, None]
    I3 = np.eye(3, dtype=np.float32)
    rots = (I3 + s * K + c * K2).reshape(b, e, 3, 3, 3).astype(np.float32)
    r = np.einsum('beij,bejk,bekl->beil', rots[:, :, 0], rots[:, :, 1],
                  rots[:, :, 2]).astype(np.float32)
    T = np.zeros((b, e, 4, 4), np.float32)
    T[..., :3, :3] = r
    T[..., 3, 3] = 1.0
    local = np.einsum('beij,ejk->beik', T,
                      tip_to_base.astype(np.float32)).astype(np.float32)
    worlds = np.zeros((b, n, 4, 4), np.float32)
    worlds[:, 0] = np.eye(4, dtype=np.float32)
    for i in range(1, n):
        par = (i - 1) // 2
        worlds[:, i] = (worlds[:, par] @ local[:, i - 1]).astype(np.float32)
    return worlds


# --------------------------------------------------------------------------- #
# device kernel build (v2: quad layout, fp16)
# --------------------------------------------------------------------------- #
def _build_nc(sc_const: float, loop_n: int = 1):
    import concourse.bacc as bacc
    import concourse.mybir as mybir
    from concourse.tile import TileContext
    import concourse.bass as bass
    from contextlib import ExitStack

    f32 = mybir.dt.float32
    f16 = mybir.dt.float16
    Alu = mybir.AluOpType
    AFT = mybir.ActivationFunctionType
    AP = bass.AP

    nc = bacc.Bacc("TRN2", target_bir_lowering=False, debug=False)

    la_d = nc.dram_tensor("la", [BPC, J], f32, kind="ExternalInput")
    tq_d = nc.dram_tensor("tq", [1, E * 48], f16, kind="ExternalInput")
    out_d = nc.dram_tensor("out", [BPC, N * 16], f32, kind="ExternalOutput")

    def eng(tag):
        return {"v": nc.vector, "g": nc.gpsimd, "a": nc.scalar}[tag]

    with TileContext(nc) as tc:
        with tc.tile_pool(name="main", bufs=1) as pool, \
             tc.tile_pool(name="ob", bufs=OB_BUFS) as obp, \
             ExitStack() as _loop_ctx:

            la_t = pool.tile([P, B4 * J], f32, name="la_t")
            th_t = pool.tile([P, J * B4], f16, name="th_t")
            ta_t = pool.tile([P, J * B4], f16, name="ta_t")
            sn_t = pool.tile([P, J * B4], f16, name="sn_t")
            cn_t = pool.tile([P, J * B4], f16, name="cn_t")
            tq_t = pool.tile([P, E * 48], f16, name="tq_t")
            lo_t = pool.tile([P, E * 48], f16, name="lo_t")
            r0_t = pool.tile([P, E * 16], f16, name="r0_t")
            r1_t = pool.tile([P, E * 16], f16, name="r1_t")
            q2_t = pool.tile([P, E * 16], f16, name="q2_t")
            tA_t = pool.tile([P, E * 16], f16, name="tA_t")
            w_t = pool.tile([P, N * 64], f16, name="w_t")
            tp_t = pool.tile([P, 64 * 48], f16, name="tp_t")
            hpi_t = pool.tile([P, 1], f32)

            def wAP(off, dims):
                a = w_t[:]
                return AP(a.tensor, a.offset + off, [list(a.ap[0])] + dims)

            def tAP(tile, off, dims):
                a = tile[:]
                return AP(a.tensor, a.offset + off, [list(a.ap[0])] + dims)

            # hoisted constants: i=3 rows of w = (0,0,0,1); pi/2 bias
            nc.vector.memset(wAP(48, [[64, N], [1, 12]]), 0.0)
            nc.vector.memset(wAP(60, [[64, N], [1, 4]]), 1.0)
            nc.gpsimd.memset(hpi_t[:], PI / 2.0)

            if loop_n > 1:
                _loop_ctx.enter_context(tc.For_i(0, loop_n, 1))

            def trig(base_t, axis, e0, ne):
                return tAP(base_t, e0 * 12 + axis * 4,
                           [[12, ne], [0, 4], [1, 4]])

            def tipv(i, e0, ne):
                return tAP(tq_t, e0 * 48 + i * 16, [[48, ne], [4, 4], [1, 4]])

            def locv(i, e0, ne):
                return tAP(lo_t, e0 * 48 + i * 16, [[48, ne], [4, 4], [1, 4]])

            def elv(tile, e0, ne):
                return tAP(tile, e0 * 16, [[16, ne], [4, 4], [1, 4]])

            def bc_stage(etag, e0, e1):
                ne = e1 - e0
                ev = eng(etag)
                sx, sy, sz = (trig(sn_t, a, e0, ne) for a in range(3))
                cx, cy, cz = (trig(cn_t, a, e0, ne) for a in range(3))
                T0, T1, T2 = (tipv(i, e0, ne) for i in range(3))
                L0, L1, L2 = (locv(i, e0, ne) for i in range(3))
                r0, r1 = elv(r0_t, e0, ne), elv(r1_t, e0, ne)
                q2, tA = elv(q2_t, e0, ne), elv(tA_t, e0, ne)
                tt = ev.tensor_tensor
                triples = [
                    (cz, T0, sz, T1, r0, Alu.subtract, True),
                    (sz, T0, cz, T1, r1, Alu.add, False),
                    (cy, r0, sy, T2, L0, Alu.add, False),
                    (sy, r0, cy, T2, q2, Alu.subtract, False),
                    (cx, r1, sx, q2, L1, Alu.subtract, True),
                    (sx, r1, cx, q2, L2, Alu.add, False),
                ]
                for (a, b, c, d, dst, op, ta_first) in triples:
                    tt(tA, a, b, Alu.mult)
                    tt(dst, c, d, Alu.mult)
                    if ta_first:
                        tt(dst, tA, dst, op)
                    else:
                        tt(dst, dst, tA, op)

            LEVELS = [(3, 7), (7, 15), (15, 31), (31, 63),
                      (63, 127), (127, 191), (191, 255)]

            def tree_level(lo, hi):
                # HW ISA allows at most 3 free dims per operand, so the muls
                # are split per output row i (wp then has no broadcast dim).
                m = hi - lo
                q = m // 2
                plo = (lo - 1) // 2
                tt = nc.vector.tensor_tensor
                for k in range(3):
                    for side in (0, 1):
                        for i in range(3):
                            wp = wAP(plo * 64 + i * 16 + k * 4,
                                     [[64, q], [0, 4], [1, 4]])
                            ls = tAP(lo_t,
                                     (lo - 1 + side) * 48 + k * 16,
                                     [[96, q], [4, 4], [1, 4]])
                            if k == 0:
                                dst = wAP((lo + side) * 64 + i * 16,
                                          [[128, q], [4, 4], [1, 4]])
                            else:
                                dst = tAP(tp_t, side * 48 + i * 16,
                                          [[96, q], [4, 4], [1, 4]])
                            tt(dst, wp, ls, Alu.mult)
                    if k > 0:
                        wdst = wAP(lo * 64, [[64, m], [16, 3], [1, 16]])
                        tmpv = tAP(tp_t, 0, [[48, m], [16, 3], [1, 16]])
                        tt(wdst, wdst, tmpv, Alu.add)
                for i in range(3):
                    wtr = wAP(lo * 64 + i * 16 + 12,
                              [[128, q], [64, 2], [1, 4]])
                    ptr = wAP(plo * 64 + i * 16 + 12,
                              [[64, q], [0, 2], [1, 4]])
                    tt(wtr, wtr, ptr, Alu.add)

            def tail255():
                tt = nc.vector.tensor_tensor
                for k in range(3):
                    wp = wAP(127 * 64 + k * 4, [[16, 3], [0, 4], [1, 4]])
                    ls = tAP(lo_t, 254 * 48 + k * 16, [[0, 3], [4, 4], [1, 4]])
                    if k == 0:
                        tt(wAP(255 * 64, [[16, 3], [4, 4], [1, 4]]), wp, ls,
                           Alu.mult)
                    else:
                        t255 = tAP(tp_t, 0, [[16, 3], [4, 4], [1, 4]])
                        tt(t255, wp, ls, Alu.mult)
                        tt(wAP(255 * 64, [[16, 3], [1, 16]]),
                           wAP(255 * 64, [[16, 3], [1, 16]]),
                           tAP(tp_t, 0, [[16, 3], [1, 16]]), Alu.add)
                tt(wAP(255 * 64 + 12, [[16, 3], [1, 4]]),
                   wAP(255 * 64 + 12, [[16, 3], [1, 4]]),
                   wAP(127 * 64 + 12, [[16, 3], [1, 4]]), Alu.add)

            def convert_chunk(c):
                b4, nb = c // 8, c % 8
                ob = obp.tile([P, 512], f32, tag="ob", name=f"ob{c}")
                src = wAP(nb * 32 * 64 + b4, [[64, 32], [16, 4], [4, 4]])
                etag = CONV_ENG[nb]
                if etag == "a":
                    nc.scalar.copy(ob[:], src)
                else:
                    eng(etag).tensor_copy(ob[:], src)
                dst = AP(out_d, b4 * 4096 + nb * 512,
                         [[4 * 4096, P], [1, 512]])
                nc.sync.dma_start(dst, ob[:])

            # ---------------- loop body ----------------
            la_v = la_d[:].rearrange("(p b) j -> p (b j)", p=P)
            for b in range(B4):
                nc.sync.dma_start(la_t[:, b * J:(b + 1) * J],
                                  la_v[:, b * J:(b + 1) * J])
            half = E * 24
            for h in range(2):
                nc.sync.dma_start(tq_t[:, h * half:(h + 1) * half],
                                  AP(tq_d, h * half, [[0, P], [1, half]]))

            la_T = tAP(la_t, 0, [[1, J], [J, B4]])
            nc.scalar.activation(th_t[:], la_T, AFT.Tanh)
            nc.scalar.activation(ta_t[:], th_t[:], AFT.Abs)
            nc.scalar.activation(sn_t[:], th_t[:], AFT.Sin, scale=sc_const)
            nc.scalar.activation(cn_t[:], ta_t[:], AFT.Sin, bias=hpi_t[:],
                                 scale=-sc_const)

            bc_stage(*BC_STAGES[0])
            bc_stage(*BC_STAGES[1])      # POOL share (late edges)

            nc.vector.memset(wAP(0, [[1, 48]]), 0.0)
            nc.vector.memset(wAP(0, [[20, 3], [1, 4]]), 1.0)
            nc.vector.tensor_copy(wAP(64, [[64, 2], [1, 48]]),
                                  tAP(lo_t, 0, [[48, 2], [1, 48]]))
            for lo, hi in LEVELS[:5]:
                tree_level(lo, hi)

            bc_stage(*BC_STAGES[2])
            tree_level(*LEVELS[5])
            tree_level(*LEVELS[6])
            tail255()
            for c in range(32):
                convert_chunk(c)

    nc.compile()
    return nc


# --------------------------------------------------------------------------- #
# cached PJRT runner (axon path) — compile once, execute per call
# --------------------------------------------------------------------------- #
def _get_runner(sc_const, loop_n=1):
    key = ("runner2", round(sc_const, 6), loop_n)
    if key in _state:
        return _state[key]

    import jax
    from jax.sharding import Mesh, PartitionSpec, NamedSharding
    from jax.experimental.shard_map import shard_map
    import concourse.mybir as mybir
    from concourse import bass2jax

    nc = _build_nc(sc_const, loop_n)
    bass2jax.install_neuronx_cc_hook()

    part_name = (nc.partition_id_tensor.name
                 if nc.partition_id_tensor is not None else None)
    in_names, out_names, out_avals = [], [], []
    for alloc in nc.m.functions[0].allocations:
        if not isinstance(alloc, mybir.MemoryLocationSet):
            continue
        name = alloc.memorylocations[0].name
        if alloc.kind == "ExternalInput":
            if name != part_name:
                in_names.append(name)
        elif alloc.kind == "ExternalOutput":
            out_names.append(name)
            out_avals.append(jax.core.ShapedArray(
                tuple(alloc.tensor_shape), mybir.dt.np(alloc.dtype)))
    n_params = len(in_names)
    all_in = in_names + out_names
    if part_name is not None:
        all_in = all_in + [part_name]

    def _body(*args):
        operands = list(args)
        if part_name is not None:
            operands.append(bass2jax.partition_id_tensor())
        outs = bass2jax._bass_exec_p.bind(
            *operands,
            out_avals=tuple(out_avals),
            in_names=tuple(all_in),
            out_names=tuple(out_names),
            lowering_input_output_aliases=(),
            sim_require_finite=True,
            sim_require_nnan=True,
            nc=nc,
        )
        return tuple(outs)

    devices = jax.devices()[:NCORE]
    mesh = Mesh(np.asarray(devices), ("core",))
    nin = n_params + len(out_names)
    sharded = jax.jit(
        shard_map(_body, mesh=mesh,
                  in_specs=(PartitionSpec("core"),) * nin,
                  out_specs=(PartitionSpec("core"),) * len(out_names),
                  check_rep=False),
        donate_argnums=tuple(range(n_params, nin)),
        keep_unused=True,
    )
    shard0 = NamedSharding(mesh, PartitionSpec("core"))

    def _make_zeros():
        return jax.jit(
            lambda: jax.numpy.zeros((NCORE * BPC, N * 16), np.float32),
            out_shardings=shard0)()

    runner = (sharded, in_names, _make_zeros)
    _state[key] = runner
    return runner


def make_feed(log_angles, tip_to_base):
    """Build the device feed dict from full inputs."""
    tip_rows = np.ascontiguousarray(
        tip_to_base[:, :3, :], dtype=np.float32)          # (E, 3, 4)
    tq = np.repeat(tip_rows.reshape(E * 12, 1), B4,
                   axis=1).reshape(1, E * 48).astype(np.float16)
    return {
        "la": np.ascontiguousarray(log_angles, dtype=np.float32),
        "tq": np.broadcast_to(tq, (NCORE, E * 48)).copy(),
    }


def _run_device(log_angles, tip_to_base, sc_const, loop_n=1):
    sharded, in_names, make_zeros = _get_runner(sc_const, loop_n)
    feed = make_feed(log_angles, tip_to_base)
    args = [feed[name] for name in in_names]
    out = sharded(*args, make_zeros())[0]
    return np.asarray(out).reshape(B, N, 4, 4)


# --------------------------------------------------------------------------- #
# public entry point
# --------------------------------------------------------------------------- #
def kernel(log_angles, tip_to_base, rot_axes, rot_constraints):
    log_angles = np.asarray(log_angles)
    tip_to_base = np.asarray(tip_to_base)
    rot_axes = np.asarray(rot_axes)
    rot_constraints = np.asarray(rot_constraints)

    expected_shapes = (log_angles.shape == (B, J)
                       and tip_to_base.shape == (E, 4, 4)
                       and rot_axes.shape == (J, 3)
                       and rot_constraints.shape == (J, 2))
    eye_tiled = np.tile(np.eye(3, dtype=np.float32), (E, 1)) \
        if expected_shapes else None
    euler = expected_shapes and np.allclose(rot_axes, eye_tiled, atol=1e-6)
    if not euler:
        return _np_fallback(log_angles, tip_to_base, rot_axes, rot_constraints)

    sc = rot_constraints[:, 0].astype(np.float32)
    of = rot_constraints[:, 1].astype(np.float32)
    const_ok = (np.all(sc == sc[0]) and np.all(of == 0.0)
                and float(sc[0]) > 1e-3
                and float(sc[0]) <= PI + 1e-4)
    if not const_ok:
        # untested-on-device parameter regime: use the exact host fallback
        return _np_fallback(log_angles, tip_to_base, rot_axes,
                            rot_constraints)

    return _run_device(log_angles, tip_to_base, float(sc[0]))
